# revision 5
# baseline (speedup 1.0000x reference)
"""Window-routed sparse attention on 8 TRN2 NeuronCores.

Sharding: 64 windows x 8 cores = 8 windows/core (embarrassingly parallel).
Host precomputes the tiny routing path (region means, a_r [64,64]) and the
window-mixed q_m/k_m in fp32 numpy; each core runs the heavy windowed
attention relu(q_m k_m^T) v for its 8 windows.

Device-side layout: windows are processed in PAIRS so the 128x128 PE array
is fully used despite c=64 contraction / c=64 output:
  - QK^T: window A occupies PE rows 0-63, window B rows 64-127
    (tile_position row packing; both matmuls run concurrently).
  - attn@V: window A drains to PSUM partitions 0-63, window B to 64-127
    (tile_position column packing).
Operands are bf16 (1 col/cycle streaming + FWL weight loads); accumulation
stays fp32 in PSUM. The relu(PSUM)->SBUF pass is load-balanced between the
Scalar (ACT) and Vector (DVE) engines, which are the throughput floor.
"""

import sys

sys.path.insert(0, "/opt/trn_rl_repo")

import numpy as np
import ml_dtypes

C = 64          # channels
NW = 64         # windows (8x8 grid of 32x32 patches on 256x256)
T = 1024        # tokens per window (32*32)
NCORES = 8
WPC = NW // NCORES  # windows per core
NPAIR = WPC // 2

_CACHE = {}


def _build_program():
    import concourse.mybir as mybir
    from concourse import bacc
    from concourse.tile import TileContext

    bf16 = mybir.dt.bfloat16
    f32 = mybir.dt.float32

    nc = bacc.Bacc(None, target_bir_lowering=False)
    # qm/km: [pair, 128, T] with partitions = (window-in-pair, c):
    #   window A channels at partitions 0-63, window B at 64-127.
    qm_d = nc.declare_dram_parameter("qm", [NPAIR, 128, T], bf16, isOutput=False)
    km_d = nc.declare_dram_parameter("km", [NPAIR, 128, T], bf16, isOutput=False)
    # v: [pair, s-in-chunk(128), window-in-pair(2), s-chunk(8), c]
    v_d = nc.declare_dram_parameter("v", [NPAIR, 128, 2, 8, C], bf16, isOutput=False)
    # o: [pair, 128, T]; partitions 0-63 = window A [c, T], 64-127 = window B
    o_d = nc.declare_dram_parameter("o", [NPAIR, 128, T], f32, isOutput=True)

    # greedy ACT/DVE load balancing for the PSUM->SBUF relu/copy passes
    eng_time = {"act": 0.0, "dve": 0.0}

    def pick_engine():
        if eng_time["act"] <= eng_time["dve"]:
            eng_time["act"] += 1040.0  # ~[128,1024] ACTIVATE ns
            return "act"
        eng_time["dve"] += 1190.0  # ~[128,1024] DVE tensor_scalar ns
        return "dve"

    def relu_to(engine, out_t, in_t):
        if engine == "act":
            nc.scalar.activation(
                out=out_t, in_=in_t,
                func=mybir.ActivationFunctionType.Relu, scale=1.0,
            )
        else:
            nc.vector.tensor_scalar_max(out_t, in_t, 0.0)

    def copy_to(engine, out_t, in_t):
        if engine == "act":
            nc.scalar.activation(
                out=out_t, in_=in_t,
                func=mybir.ActivationFunctionType.Copy, scale=1.0,
            )
        else:
            nc.vector.tensor_copy(out=out_t, in_=in_t)

    with TileContext(nc) as tc:
        with (
            tc.tile_pool(name="qk", bufs=2) as qk_pool,
            tc.tile_pool(name="vp", bufs=2) as v_pool,
            tc.tile_pool(name="at", bufs=3) as a_pool,
            tc.tile_pool(name="ob", bufs=2) as o_pool,
            tc.tile_pool(name="wm", bufs=1) as w_pool,
            # 3 rotating [128,T] QK-output tiles (6 banks) + 1 ps_o (2 banks)
            tc.tile_pool(name="pa", bufs=3, space="PSUM") as pa_pool,
            tc.tile_pool(name="po", bufs=1, space="PSUM") as po_pool,
        ):
            # preload the ACT Relu table while the first DMAs are in flight
            warm_t = w_pool.tile([1, 2], f32, tag="warm")
            nc.vector.memset(warm_t, 0.0)
            nc.scalar.activation(
                out=warm_t, in_=warm_t,
                func=mybir.ActivationFunctionType.Relu, scale=1.0,
            )

            for p in range(NPAIR):
                qm_t = qk_pool.tile([128, T], bf16, tag="qm")
                km_t = qk_pool.tile([128, T], bf16, tag="km")
                v_t = v_pool.tile([128, 2, 8, C], bf16, tag="v")
                # chunked so the first QK pair starts after ~200KB, not 768KB
                nc.sync.dma_start(out=qm_t[:, 0:512], in_=qm_d[p, :, 0:512])
                nc.sync.dma_start(out=km_t[:, 0:256], in_=km_d[p, :, 0:256])
                nc.sync.dma_start(out=qm_t[:, 512:T], in_=qm_d[p, :, 512:T])
                nc.sync.dma_start(out=km_t[:, 256:T], in_=km_d[p, :, 256:T])
                nc.sync.dma_start(out=v_t, in_=v_d[p])

                def qk_pair(k):
                    # QK^T for s-chunk k: two windows row-packed on the PE
                    # array (A on rows 0-63, B on rows 64-127, concurrent)
                    ks = slice(k * 128, (k + 1) * 128)
                    ps_a = pa_pool.tile([128, T], f32, tag="ps")
                    ps_b = pa_pool.tile([128, T], f32, tag="ps")
                    for h in range(2):
                        hs = slice(h * 512, (h + 1) * 512)
                        nc.tensor.matmul(
                            out=ps_a[:, hs], lhsT=km_t[0:64, ks],
                            rhs=qm_t[0:64, hs], start=True, stop=True,
                        )
                        nc.tensor.matmul(
                            out=ps_b[:, hs], lhsT=km_t[64:128, ks],
                            rhs=qm_t[64:128, hs], start=True, stop=True,
                        )
                    return ps_a, ps_b

                ps_o = po_pool.tile([128, T], f32, tag="pso")
                # 2-deep QK lookahead: the PE stream per cycle is
                # [AV(k), QK(k+2)], so the relu engines never wait on a QK
                # that is queued behind an AV (head-of-line blocking).
                pend = [qk_pair(0), qk_pair(1)]
                for k in range(8):
                    ps_a, ps_b = pend.pop(0)
                    at_a = a_pool.tile([128, T], bf16, tag="at_a")
                    at_b = a_pool.tile([128, T], bf16, tag="at_b")
                    relu_to(pick_engine(), at_a, ps_a)
                    relu_to(pick_engine(), at_b, ps_b)
                    # attn @ V: two windows column-packed (A -> psum partitions
                    # 0-63, B -> 64-127), accumulating over s-chunks k
                    for h in range(2):
                        hs = slice(h * 512, (h + 1) * 512)
                        nc.tensor.matmul(
                            out=ps_o[0:64, hs], lhsT=v_t[:, 0, k, :],
                            rhs=at_a[:, hs], start=(k == 0), stop=(k == 7),
                        )
                        nc.tensor.matmul(
                            out=ps_o[64:128, hs], lhsT=v_t[:, 1, k, :],
                            rhs=at_b[:, hs], start=(k == 0), stop=(k == 7),
                        )
                    if k + 2 < 8:
                        pend.append(qk_pair(k + 2))
                o_t = o_pool.tile([128, T], f32, tag="o")
                copy_to(pick_engine(), o_t, ps_o)
                nc.sync.dma_start(out=o_d[p], in_=o_t)

    nc.finalize()
    return nc


def kernel(x, W, bias):
    from concourse.bass_utils import run_bass_kernel_spmd

    x = np.asarray(x, dtype=np.float32)
    W = np.asarray(W, dtype=np.float32)
    bias = np.asarray(bias, dtype=np.float32)

    # ---- host prep: windows, qkv, routing, mixing (tiny vs attention) ----
    # xw: [nw, T, c]
    xw = (
        x.reshape(C, 8, 32, 8, 32)
        .transpose(1, 3, 2, 4, 0)
        .reshape(NW, T, C)
    )
    qkv = xw @ W.T + bias  # [nw, T, 3c]
    q, k, v = qkv[..., :C], qkv[..., C:2 * C], qkv[..., 2 * C:]
    q_r = q.mean(axis=1)  # [nw, c]
    k_r = k.mean(axis=1)
    a_r = np.maximum(q_r @ k_r.T, 0.0)  # [nw, nw]
    k_m = np.tensordot(a_r, k, axes=(1, 0))  # [nw, T, c]
    q_m = np.tensordot(a_r, q, axes=(1, 0))

    if "nc" not in _CACHE:
        _CACHE["nc"] = _build_program()
    nc = _CACHE["nc"]

    bf16 = ml_dtypes.bfloat16
    in_maps = []
    for m in range(NCORES):
        s = slice(m * WPC, (m + 1) * WPC)
        # [wpc, T, c] -> [pair, 2, c, T] -> [pair, 128, T]
        qm_p = (
            q_m[s].reshape(NPAIR, 2, T, C).transpose(0, 1, 3, 2)
            .reshape(NPAIR, 128, T)
        )
        km_p = (
            k_m[s].reshape(NPAIR, 2, T, C).transpose(0, 1, 3, 2)
            .reshape(NPAIR, 128, T)
        )
        # [wpc, T, c] -> [pair, 2, 8, 128, c] -> [pair, 128, 2, 8, c]
        v_p = v[s].reshape(NPAIR, 2, 8, 128, C).transpose(0, 3, 1, 2, 4)
        in_maps.append({
            "qm": np.ascontiguousarray(qm_p).astype(bf16),
            "km": np.ascontiguousarray(km_p).astype(bf16),
            "v": np.ascontiguousarray(v_p).astype(bf16),
        })

    res = run_bass_kernel_spmd(nc, in_maps, list(range(NCORES)))
    _CACHE["last_res"] = res
    # [pair, 128, T] -> [wpc, c, T] per core
    outs = [
        res.results[m]["o"].reshape(NPAIR * 2, C, T) for m in range(NCORES)
    ]
    o_all = np.concatenate(outs, axis=0)  # [nw, c, T]
    o_cm = o_all.transpose(1, 0, 2)  # [c, nw, T]

    # fold back: [c, jh, jw, th, tw] -> [1, c, 256, 256]
    o_img = (
        o_cm.reshape(C, 8, 8, 32, 32)
        .transpose(0, 1, 3, 2, 4)
        .reshape(1, C, 256, 256)
    )
    return o_img.astype(np.float32)


# revision 7
# speedup vs baseline: 1.1781x; 1.1781x over previous
"""Window-routed sparse attention on 8 TRN2 NeuronCores.

Sharding: 64 windows x 8 cores = 8 windows/core (embarrassingly parallel).
Host precomputes the tiny routing path (region means, a_r [64,64]) and the
window-mixed q_m/k_m in fp32 numpy; each core runs the heavy windowed
attention relu(q_m k_m^T) v for its 8 windows.

Device-side layout: windows are processed in PAIRS so the 128x128 PE array
is fully used despite c=64 contraction / c=64 output:
  - QK^T: window A occupies PE rows 0-63, window B rows 64-127
    (tile_position row packing; both matmuls run concurrently).
  - attn@V: window A drains to PSUM partitions 0-63, window B to 64-127
    (tile_position column packing).
Operands are bf16 (1 col/cycle streaming + FWL weight loads); accumulation
stays fp32 in PSUM. The relu(PSUM)->SBUF pass is load-balanced between the
Scalar (ACT) and Vector (DVE) engines, which are the throughput floor.
"""

import sys

sys.path.insert(0, "/opt/trn_rl_repo")

import numpy as np
import ml_dtypes

C = 64          # channels
NW = 64         # windows (8x8 grid of 32x32 patches on 256x256)
T = 1024        # tokens per window (32*32)
NCORES = 8
WPC = NW // NCORES  # windows per core
NPAIR = WPC // 2

_CACHE = {}


def _build_program():
    import concourse.mybir as mybir
    from concourse import bacc
    from concourse.tile import TileContext

    bf16 = mybir.dt.bfloat16
    f32 = mybir.dt.float32

    nc = bacc.Bacc(None, target_bir_lowering=False)
    # qm/km: [pair, 128, T] with partitions = (window-in-pair, c):
    #   window A channels at partitions 0-63, window B at 64-127.
    qm_d = nc.declare_dram_parameter("qm", [NPAIR, 128, T], bf16, isOutput=False)
    km_d = nc.declare_dram_parameter("km", [NPAIR, 128, T], bf16, isOutput=False)
    # v: [pair, s-in-chunk(128), window-in-pair(2), s-chunk(8), c]
    v_d = nc.declare_dram_parameter("v", [NPAIR, 128, 2, 8, C], bf16, isOutput=False)
    # o: [pair, 128, T]; partitions 0-63 = window A [c, T], 64-127 = window B
    o_d = nc.declare_dram_parameter("o", [NPAIR, 128, T], f32, isOutput=True)

    # greedy ACT/DVE load balancing for the PSUM->SBUF relu/copy passes
    eng_time = {"act": 0.0, "dve": 0.0}

    def pick_engine():
        if eng_time["act"] <= eng_time["dve"]:
            eng_time["act"] += 1040.0  # ~[128,1024] ACTIVATE ns
            return "act"
        eng_time["dve"] += 1190.0  # ~[128,1024] DVE tensor_scalar ns
        return "dve"

    def relu_to(engine, out_t, in_t):
        if engine == "act":
            nc.scalar.activation(
                out=out_t, in_=in_t,
                func=mybir.ActivationFunctionType.Relu, scale=1.0,
            )
        else:
            nc.vector.tensor_scalar_max(out_t, in_t, 0.0)

    def copy_to(engine, out_t, in_t):
        if engine == "act":
            nc.scalar.activation(
                out=out_t, in_=in_t,
                func=mybir.ActivationFunctionType.Copy, scale=1.0,
            )
        else:
            nc.vector.tensor_copy(out=out_t, in_=in_t)

    with TileContext(nc) as tc:
        with (
            tc.tile_pool(name="qk", bufs=2) as qk_pool,
            tc.tile_pool(name="vp", bufs=2) as v_pool,
            tc.tile_pool(name="at", bufs=2) as a_pool,
            tc.tile_pool(name="ob", bufs=2) as o_pool,
            tc.tile_pool(name="wm", bufs=1) as w_pool,
            # 3 rotating [128,T] QK-output tiles (6 banks) + 1 ps_o (2 banks)
            tc.tile_pool(name="pa", bufs=3, space="PSUM") as pa_pool,
            tc.tile_pool(name="po", bufs=1, space="PSUM") as po_pool,
        ):
            # preload the ACT Relu table while the first DMAs are in flight
            warm_t = w_pool.tile([1, 2], f32, tag="warm")
            nc.vector.memset(warm_t, 0.0)
            nc.scalar.activation(
                out=warm_t, in_=warm_t,
                func=mybir.ActivationFunctionType.Relu, scale=1.0,
            )

            for p in range(NPAIR):
                qm_t = qk_pool.tile([128, T], bf16, tag="qm")
                km_t = qk_pool.tile([128, T], bf16, tag="km")
                v_t = v_pool.tile([128, 2, 8, C], bf16, tag="v")
                # chunked so the first QK pair starts after ~200KB, not 768KB
                nc.sync.dma_start(out=qm_t[:, 0:512], in_=qm_d[p, :, 0:512])
                nc.sync.dma_start(out=km_t[:, 0:256], in_=km_d[p, :, 0:256])
                nc.sync.dma_start(out=qm_t[:, 512:T], in_=qm_d[p, :, 512:T])
                nc.sync.dma_start(out=km_t[:, 256:T], in_=km_d[p, :, 256:T])
                nc.sync.dma_start(out=v_t, in_=v_d[p])

                def qk_pair(k):
                    # QK^T for s-chunk k: two windows row-packed on the PE
                    # array (A on rows 0-63, B on rows 64-127, concurrent)
                    ks = slice(k * 128, (k + 1) * 128)
                    ps_a = pa_pool.tile([128, T], f32, tag="ps")
                    ps_b = pa_pool.tile([128, T], f32, tag="ps")
                    for h in range(2):
                        hs = slice(h * 512, (h + 1) * 512)
                        nc.tensor.matmul(
                            out=ps_a[:, hs], lhsT=km_t[0:64, ks],
                            rhs=qm_t[0:64, hs], start=True, stop=True,
                        )
                        nc.tensor.matmul(
                            out=ps_b[:, hs], lhsT=km_t[64:128, ks],
                            rhs=qm_t[64:128, hs], start=True, stop=True,
                        )
                    return ps_a, ps_b

                ps_o = po_pool.tile([128, T], f32, tag="pso")
                cur = qk_pair(0)
                for k in range(8):
                    ps_a, ps_b = cur
                    at_a = a_pool.tile([128, T], bf16, tag="at_a")
                    at_b = a_pool.tile([128, T], bf16, tag="at_b")
                    relu_to(pick_engine(), at_a, ps_a)
                    relu_to(pick_engine(), at_b, ps_b)
                    # keep the PE stream dense: QK(k+1) issues before AV(k)
                    if k < 7:
                        cur = qk_pair(k + 1)
                    # attn @ V: two windows column-packed (A -> psum partitions
                    # 0-63, B -> 64-127), accumulating over s-chunks k
                    for h in range(2):
                        hs = slice(h * 512, (h + 1) * 512)
                        nc.tensor.matmul(
                            out=ps_o[0:64, hs], lhsT=v_t[:, 0, k, :],
                            rhs=at_a[:, hs], start=(k == 0), stop=(k == 7),
                        )
                        nc.tensor.matmul(
                            out=ps_o[64:128, hs], lhsT=v_t[:, 1, k, :],
                            rhs=at_b[:, hs], start=(k == 0), stop=(k == 7),
                        )
                o_t = o_pool.tile([128, T], f32, tag="o")
                copy_to(pick_engine(), o_t, ps_o)
                nc.sync.dma_start(out=o_d[p], in_=o_t)

    nc.finalize()
    return nc


def kernel(x, W, bias):
    from concourse.bass_utils import run_bass_kernel_spmd

    x = np.asarray(x, dtype=np.float32)
    W = np.asarray(W, dtype=np.float32)
    bias = np.asarray(bias, dtype=np.float32)

    # ---- host prep: windows, qkv, routing, mixing (tiny vs attention) ----
    # xw: [nw, T, c]
    xw = (
        x.reshape(C, 8, 32, 8, 32)
        .transpose(1, 3, 2, 4, 0)
        .reshape(NW, T, C)
    )
    qkv = xw @ W.T + bias  # [nw, T, 3c]
    q, k, v = qkv[..., :C], qkv[..., C:2 * C], qkv[..., 2 * C:]
    q_r = q.mean(axis=1)  # [nw, c]
    k_r = k.mean(axis=1)
    a_r = np.maximum(q_r @ k_r.T, 0.0)  # [nw, nw]
    k_m = np.tensordot(a_r, k, axes=(1, 0))  # [nw, T, c]
    q_m = np.tensordot(a_r, q, axes=(1, 0))

    if "nc" not in _CACHE:
        _CACHE["nc"] = _build_program()
    nc = _CACHE["nc"]

    bf16 = ml_dtypes.bfloat16
    in_maps = []
    for m in range(NCORES):
        s = slice(m * WPC, (m + 1) * WPC)
        # [wpc, T, c] -> [pair, 2, c, T] -> [pair, 128, T]
        qm_p = (
            q_m[s].reshape(NPAIR, 2, T, C).transpose(0, 1, 3, 2)
            .reshape(NPAIR, 128, T)
        )
        km_p = (
            k_m[s].reshape(NPAIR, 2, T, C).transpose(0, 1, 3, 2)
            .reshape(NPAIR, 128, T)
        )
        # [wpc, T, c] -> [pair, 2, 8, 128, c] -> [pair, 128, 2, 8, c]
        v_p = v[s].reshape(NPAIR, 2, 8, 128, C).transpose(0, 3, 1, 2, 4)
        in_maps.append({
            "qm": np.ascontiguousarray(qm_p).astype(bf16),
            "km": np.ascontiguousarray(km_p).astype(bf16),
            "v": np.ascontiguousarray(v_p).astype(bf16),
        })

    res = run_bass_kernel_spmd(nc, in_maps, list(range(NCORES)))
    _CACHE["last_res"] = res
    # [pair, 128, T] -> [wpc, c, T] per core
    outs = [
        res.results[m]["o"].reshape(NPAIR * 2, C, T) for m in range(NCORES)
    ]
    o_all = np.concatenate(outs, axis=0)  # [nw, c, T]
    o_cm = o_all.transpose(1, 0, 2)  # [c, nw, T]

    # fold back: [c, jh, jw, th, tw] -> [1, c, 256, 256]
    o_img = (
        o_cm.reshape(C, 8, 8, 32, 32)
        .transpose(0, 1, 3, 2, 4)
        .reshape(1, C, 256, 256)
    )
    return o_img.astype(np.float32)
